# revision 59
# baseline (speedup 1.0000x reference)
"""Cross-attention (B=2, N=M=2048, DIM=1024, H=16) on 8 TRN2 NeuronCores, v5.

Sharding: tensor-parallel over heads (2 heads/core). Key structure vs v3:
  - score matmuls for the two heads issued adjacently as 64-row PE tiles
    ((0,0)/(64,0), auto-inferred from base partitions) so they stream
    concurrently on the PE (~512 cyc/pair instead of ~2x700).
  - per-head exp on BOTH engines in parallel every mt (ScalarE LUT exp
    with the Schraudolph multiply folded into Wq host-side, scale=ln2/128
    restores e^x; VectorE 1-ALU-op Schraudolph add -> int16 bitcast bf16):
    ~0.65us latency decouples the score->exp->AV chain from the beat pace.
  - 2-mt beats with AV lag 3 ([AV(2i-4) AV(2i-3) fillers][S(2i) S(2i+1)]),
    score psum singles bufs=5.
  - AV keeps the D+1 ones-column (M=65): den via any other route costs
    ~95us of DVE/GpSimd adds (measured rates), worse than the half-array
    matmul waste.
  - skew-proof schedule: run-to-run inter-core launch skew is 10-70us and
    is absorbed by the collectives; every of-load/D-group lands >=2 qbs
    (~75us) after its coll fires so the PE FIFO never stalls mid-stream;
    fillers carry a scheduling-only pin (add_dep_helper sync=False) to the
    previous qb's last AV so the Tile scheduler cannot hoist them earlier.
    Only coll3 stays on the critical path; D2 fills its wait window (and
    re-ramps the PE pstate for D3).
  - ata_in DMAs ride the gpsimd queue (right before their trigger); of
    loads are emitted just before their consumer so their coll-wait never
    blocks the sync-queue FIFO behind x loads.
  - x2 ch0/ch1 split per-k so the first k-matmul starts after 64KB under
    the all-core start-of-kernel HBM burst; b1 x loads staggered.
  - front-loaded B phase: only k(all) + q(0,0) + v(ch0) precede C; v
    chunks 1-3 and q(0,1..3) run as (0,0) fillers, starting C ~12us
    earlier.
  - output stored/DMA'd bf16 per 512-col half (f32-ified host-side).

Compute dtype: bf16 matmul operands, f32 PSUM accumulation.
"""

import sys

for _p in ("/opt/trn_rl_repo",):
    if _p not in sys.path:
        sys.path.append(_p)

import math

import ml_dtypes
import numpy as np

import concourse.bass as bass
import concourse.mybir as mybir
import concourse.tile as tile
from concourse.tile import add_dep_helper
from concourse import bacc

NCORES = 8
B, N, M, DIM, H = 2, 2048, 2048, 1024, 16
D = DIM // H                  # 64 head dim
HPC = H // NCORES             # 2 heads per core
DLOC = HPC * D                # 128 local q/k/v dims per core
TOK = B * N                   # 4096 query tokens
NB = 512                      # token chunk / psum bank width (f32)
KT = DIM // 128               # 8 contraction tiles for projections
MT = M // 128                 # 16 m-tiles per batch
NQB = N // NB                 # 4 query blocks per batch
NCH = TOK // NB               # 8 token chunks total
NCOLL = 4                     # collectives (2 chunks = 1024 tokens each)
TSL = TOK // NCORES           # 512 output tokens per core
SCALE = float(D) ** -0.5

# Schraudolph constants. The 128*log2(e) multiply is folded into Wq
# host-side, so scores arrive as s_pre = 128*log2e*(q.k*SCALE):
#   DVE:    bits16 = s_pre + C_SCH  (1 ALU op, truncating f32->i16)
#   Scalar: e^x = exp(s_pre * ln2/128) via the activation's free affine
K_SCH = 128.0 * math.log2(math.e)
C_SCH = 128.0 * (127.0 - 0.0436775) + 0.5   # +0.5 assumes truncating convert
EXP_SCALE = math.log(2.0) / 128.0

BF16 = mybir.dt.bfloat16
F32 = mybir.dt.float32
I16 = mybir.dt.int16
AF = mybir.ActivationFunctionType
ALU = mybir.AluOpType


def build():
    nc = bacc.Bacc("TRN2", target_bir_lowering=False, debug=False,
                   num_devices=NCORES)

    # host-pre-tiled inputs: x?t[ch] is one contiguous [128, KT, NB] block
    x1t = nc.declare_dram_parameter("x1t", [NCH, 128, KT, NB], BF16,
                                    isOutput=False)
    x2t = nc.declare_dram_parameter("x2t", [NCH, 128, KT, NB], BF16,
                                    isOutput=False)
    wq = nc.declare_dram_parameter("wq", [128, KT, DLOC], BF16, isOutput=False)
    wk = nc.declare_dram_parameter("wk", [128, KT, DLOC], BF16, isOutput=False)
    wv = nc.declare_dram_parameter("wv", [128, KT, DLOC], BF16, isOutput=False)
    wp = nc.declare_dram_parameter("wp", [128, KT, DIM], BF16, isOutput=False)
    bp = nc.declare_dram_parameter("bp", [1, DIM], BF16, isOutput=False)
    out = nc.declare_dram_parameter("out", [TSL, DIM], BF16, isOutput=True)

    # DRAM bounce buffers for the 4 chunked AllToAlls
    ata_in = [nc.dram_tensor(f"ata_in{j}", [NCORES, DLOC, 128], BF16)
              for j in range(NCOLL)]
    ata_out = [nc.dram_tensor(f"ata_out{j}", [NCORES, DLOC, 128], BF16)
               for j in range(NCOLL)]

    with tile.TileContext(nc) as tc:
        with (
            tc.tile_pool(name="persist", bufs=1) as pp,
            tc.tile_pool(name="xin", bufs=3) as xp,
            tc.tile_pool(name="ptb", bufs=10) as ptp,      # bf16 exp out (ACT)
            tc.tile_pool(name="pti", bufs=10) as ptip,     # int16 exp out (DVE)
            tc.tile_pool(name="norm", bufs=4) as np_,
            tc.tile_pool(name="yout", bufs=2) as yp,
            tc.tile_pool(name="ofp", bufs=1) as ofp,
        ):
            # ---- persistent SBUF tensors ----
            wq_sb = pp.tile([128, KT, DLOC], BF16, tag="wq")
            wk_sb = pp.tile([128, KT, DLOC], BF16, tag="wk")
            wv_sb = pp.tile([128, KT, DLOC], BF16, tag="wv")
            wp_sb = pp.tile([128, KT, DIM], BF16, tag="wp")
            bp_sb = pp.tile([1, DIM], BF16, tag="bp")
            bias_bc = pp.tile([128, DIM], BF16, tag="bias_bc")
            qt_b = [pp.tile([128, N], BF16, tag=f"qt{b}", name=f"qt{b}")
                    for b in range(B)]
            kt_b = [pp.tile([128, M], BF16, tag=f"kt{b}", name=f"kt{b}")
                    for b in range(B)]
            v_b = [pp.tile([128, MT, HPC, D + 1], BF16, tag=f"v{b}",
                           name=f"v{b}")
                   for b in range(B)]
            # normalized head-output, indexed [dloc, coll, dest_seg, 128tok]
            ot_sb = pp.tile([128, NCOLL, NCORES, 128], BF16, tag="ot")

            # wk per-k on sync ahead of the x pieces: the first k-matmul
            # fires on wk[0]+x2[0,k0] (~8.5us) instead of the whole-tensor
            # wk DMA (~10.2us); wv moves to the gpsimd queue front
            for k in range(KT):
                nc.sync.dma_start(wk_sb[:, k, :], wk[:, k, :])
            nc.gpsimd.dma_start(wv_sb[:], wv[:])
            nc.scalar.dma_start(wq_sb[:], wq[:])
            nc.scalar.dma_start(wp_sb[:], wp[:])
            nc.gpsimd.dma_start(bp_sb[:], bp[:])
            nc.gpsimd.partition_broadcast(bias_bc[:], bp_sb[0:1, :])
            ones_sb = pp.tile([1, D], F32, tag="ones")
            nc.vector.memset(ones_sb[:], 1.0)
            for b in range(B):
                nc.vector.memset(v_b[b][:, :, :, D], 1.0)

            # x DMAs (sync queue): all of batch-0 x2 first (C(b0) needs full
            # k/v), then batch-0 x1, then batch 1.
            x2_tiles = {}
            x1_tiles = {}

            def load_x2(ch, split=False):
                t = xp.tile([128, KT, NB], BF16, tag="x2", bufs=5, name="x2t")
                if split:
                    # per-k pieces: the first k-matmul starts after 64KB
                    # instead of the full 512KB chunk (start-of-kernel
                    # HBM is contended by all 8 cores' x loads)
                    for k in range(KT):
                        nc.sync.dma_start(t[:, k, :], x2t[ch, :, k, :])
                else:
                    nc.sync.dma_start(t[:], x2t[ch])
                x2_tiles[ch] = t

            def load_x1(ch):
                t = xp.tile([128, KT, NB], BF16, tag="x1", bufs=4, name="x1t")
                nc.sync.dma_start(t[:], x1t[ch])
                x1_tiles[ch] = t

            for ch in range(NQB):          # batch 0
                load_x2(ch, split=(ch < 2))
            for ch in range(NQB):
                load_x1(ch)

            with (
                tc.tile_pool(name="ps_b", bufs=1, space="PSUM") as psb,
                tc.tile_pool(name="ps_s", bufs=5, space="PSUM") as pss,
                tc.tile_pool(name="ps_o", bufs=1, space="PSUM") as pso,
            ):
                # score psum tiles double as B-phase scratch for b0's
                # k/v/q units.
                def s_alloc():
                    t = pss.tile([128, NB], F32, tag="s", name="s_ps")
                    return t[:]

                # ---------- phase B pieces ----------
                # pre-C units (batch 0) evacuate psum via the idle ScalarE
                # and draw psum from the score-pair pool; in-C filler units
                # (batch 1) use VectorE + the 1-bank B pool.
                def emit_k_unit(b, i, alloc, eng):
                    k_ps = alloc()
                    x2_t = x2_tiles[NQB * b + i]
                    for k in range(KT):
                        nc.tensor.matmul(k_ps, wk_sb[:, k, :],
                                         x2_t[:, k, :],
                                         start=(k == 0), stop=(k == KT - 1))
                    _copy(eng, kt_b[b][:, NB * i:NB * (i + 1)], k_ps)

                def emit_v_units(b, i, alloc, eng):
                    state = {}

                    def vj(j):
                        if j == 0:
                            state["v_ps"] = alloc()
                        v_ps = state["v_ps"]
                        x2_t = x2_tiles[NQB * b + i]
                        for k in range(KT):
                            nc.tensor.matmul(
                                v_ps[:, 128 * j:128 * j + DLOC],
                                x2_t[:, k, 128 * j:128 * (j + 1)],
                                wv_sb[:, k, :],
                                start=(k == 0), stop=(k == KT - 1))
                        if j == 3:
                            for hh in range(HPC):
                                _copy(eng,
                                      v_b[b][:, 4 * i:4 * i + 4, hh, 0:D],
                                      v_ps.rearrange(
                                          "p (j d) -> p j d", j=4)[
                                          :, :, D * hh:D * (hh + 1)])
                    return [lambda j=j: vj(j) for j in range(4)]

                def emit_q_unit(b, i, alloc, eng):
                    q_ps = alloc()
                    x1_t = x1_tiles[NQB * b + i]
                    for k in range(KT):
                        nc.tensor.matmul(q_ps, wq_sb[:, k, :],
                                         x1_t[:, k, :],
                                         start=(k == 0), stop=(k == KT - 1))
                    _copy(eng, qt_b[b][:, NB * i:NB * (i + 1)], q_ps)

                def b_alloc():
                    t = psb.tile([128, NB], F32, tag="bps", name="b_ps")
                    return t[:]

                def _copy(eng, dst, src):
                    if eng is nc.scalar:
                        nc.scalar.copy(dst, src)
                    else:
                        nc.vector.tensor_copy(dst, src)

                # ---------- phase D pieces ----------
                # scheduling-only pin: stops the Tile scheduler from
                # hoisting D matmuls ahead of earlier qbs, where they
                # stall the PE FIFO on not-yet-arrived collectives (its
                # cost model doesn't know colls absorb ~25us launch skew)
                _pin = [None, None]   # [anchor, last AV inst]
                _last_d = [None]      # last D matmul emitted

                def _pin_to(inst):
                    if _pin[0] is not None and inst is not None:
                        add_dep_helper(inst.ins, _pin[0].ins, sync=False,
                                       reason="pin filler to host qb")

                of_tiles = {}

                def emit_of_load(j):
                    of = ofp.tile([128, NCORES, 128], BF16, tag=f"of{j}",
                                  name=f"of{j}")
                    nc.sync.dma_start(
                        of[:], ata_out[j][:].rearrange("s p t -> p s t"))
                    of_tiles[j] = of

                def emit_d_units(j):
                    units = []
                    box = {}

                    def mk(eb, k):
                        def mm():
                            if eb == 0 and k == 0:
                                box["y"] = yp.tile([128, DIM], BF16,
                                                   tag="ysb", name="y_sb")
                            if k == 0:
                                box[f"ps{eb}"] = psb.tile(
                                    [128, NB], F32, tag="bps", name="y_ps")
                            y_ps = box[f"ps{eb}"]
                            esl = slice(NB * eb, NB * (eb + 1))
                            mm_i = nc.tensor.matmul(
                                y_ps[:], of_tiles[j][:, k, :],
                                wp_sb[:, k, esl],
                                start=(k == 0),
                                stop=(k == NCORES - 1))
                            if k == 0:
                                _pin_to(mm_i)
                            _last_d[0] = mm_i
                            if k == NCORES - 1:
                                nc.vector.tensor_add(
                                    box["y"][:, esl], y_ps[:],
                                    bias_bc[:, esl])
                                nc.sync.dma_start(
                                    out[128 * j:128 * (j + 1), esl],
                                    box["y"][:, esl])
                        return mm

                    for eb in range(DIM // NB):
                        for k in range(NCORES):
                            units.append(mk(eb, k))
                    return units

                # ---------- phase C ----------
                def emit_c_qb(b, qb, fillers):
                    ch = NQB * b + qb
                    lnsl = slice(NB * qb, NB * (qb + 1))
                    # fillers may not schedule before the previous qb's AV
                    # chain has finished (keeps D matmuls, which wait on
                    # collectives, out of earlier qbs' PE streams)
                    _pin[0] = _pin[1]
                    o_ps = [pso.tile([D + 1, NB], F32, tag=f"ops{hh}",
                                     name=f"o_ps{hh}")
                            for hh in range(HPC)]
                    pts = []

                    def scores(mt):
                        msl = slice(128 * mt, 128 * (mt + 1))
                        # adjacent matmuls -> auto PE row tiles (0,0)/(64,0)
                        # run concurrently; separate banks. Per-head exp on
                        # BOTH engines in parallel (low latency decouples
                        # the score->exp->AV chain from the beat cadence).
                        sp = [pss.tile([128, NB], F32, tag="s", name="s_ps")
                              for _ in range(HPC)]
                        for hh in range(HPC):
                            hsl = slice(D * hh, D * (hh + 1))
                            nc.tensor.matmul(
                                sp[hh][:], kt_b[b][hsl, msl],
                                qt_b[b][hsl, lnsl],
                                start=True, stop=True)
                        pt0 = ptp.tile([128, NB], BF16, tag="ptb",
                                       name="pt_b")
                        nc.scalar.activation(pt0[:], sp[0][:],
                                             AF.Exp, scale=EXP_SCALE)
                        pt1 = ptip.tile([128, NB], I16, tag="pti",
                                        name="pt_i")
                        nc.vector.tensor_scalar(pt1[:], sp[1][:],
                                                C_SCH, None, ALU.add)
                        pts.append([pt0[:], pt1[:].bitcast(BF16)])

                    def av(mt):
                        for hh in range(HPC):
                            mm_i = nc.tensor.matmul(
                                o_ps[hh][:], v_b[b][:, mt, hh, :],
                                pts[mt][hh],
                                start=(mt == 0), stop=(mt == MT - 1))
                            _pin[1] = mm_i

                    def pop_filler():
                        if fillers:
                            fillers.pop(0)()

                    # beats with AV lag 3: [AV(2i-4) AV(2i-3) f..][S(2i) S(2i+1)]
                    # (4-mt same-mode groups measured WORSE despite fewer
                    # tiling-mode switches - longer AV runs re-couple the
                    # exp->AV chain to the beat cadence)
                    scores(0)
                    scores(1)
                    scores(2)
                    scores(3)
                    for i in range(2, MT // 2):
                        av(2 * i - 4)
                        pop_filler()
                        av(2 * i - 3)
                        pop_filler()
                        pop_filler()
                        scores(2 * i)
                        scores(2 * i + 1)
                    for mt in range(MT - 4, MT):
                        av(mt)
                        pop_filler()
                    while fillers:
                        fillers.pop(0)()
                    # normalize: 1/den straight from psum (fast approx),
                    # broadcast via a K=1 PE matmul (GpSimd partition
                    # broadcast costs ~1us), multiply psum x psum -> ot.
                    # h1 first: o_ps1 must clear before next qb's AV.
                    j, par = ch // 2, ch % 2
                    for hh in (1, 0):
                        hsl = slice(D * hh, D * (hh + 1))
                        dn = np_.tile([1, NB], F32, tag="den", name="dn")
                        nc.scalar.copy(dn[:], o_ps[hh][D:D + 1, :])
                        rc = np_.tile([1, NB], F32, tag="recip", name="rc")
                        nc.vector.reciprocal_approx_fast(rc[:], dn[:])
                        bc = np_.tile([D, NB], F32, tag="bcast", name="bc")
                        nc.gpsimd.partition_broadcast(bc[:], rc[0:1, :])
                        nc.vector.tensor_mul(
                            ot_sb[hsl, j, 4 * par:4 * par + 4, :].rearrange(
                                "p a t -> p (a t)"),
                            o_ps[hh][0:D, :], bc[:])

                def emit_coll(j):
                    # gpsimd queue: lands right before its own trigger and
                    # bypasses the sync FIFO (x loads would delay it there)
                    nc.gpsimd.dma_start(
                        ata_in[j][:].rearrange("s p t -> p s t"),
                        ot_sb[:, j, :, :])
                    nc.gpsimd.collective_compute(
                        "AllToAll", mybir.AluOpType.bypass,
                        replica_groups=[list(range(NCORES))],
                        ins=[ata_in[j].ap().opt()],
                        outs=[ata_out[j].ap().opt()],
                    )

                # ---------- schedule ----------
                # front-load only what (0,0) needs to START (k all, q(0,0),
                # v chunk 0); v chunks 1-3 become (0,0) fillers so the C
                # pipeline begins ~12us earlier
                for i in range(NQB):
                    emit_k_unit(0, i, s_alloc, nc.scalar)
                emit_q_unit(0, 0, s_alloc, nc.scalar)
                for u in emit_v_units(0, 0, s_alloc, nc.scalar):
                    u()

                def b1_kv(i):
                    return ([lambda i=i: emit_k_unit(1, i, b_alloc,
                                                     nc.vector)]
                            + emit_v_units(1, i, b_alloc, nc.vector))

                def b1_q(i):
                    return [lambda i=i: emit_q_unit(1, i, b_alloc,
                                                    nc.vector)]

                # b1 x loads staggered to flatten the HBM spike (full x
                # loads on all 8 cores saturate HBM and skew the cores,
                # which the collectives then pay for as barrier waits)
                load_x2(4)
                load_x2(5)
                emit_c_qb(0, 0, (
                    emit_v_units(0, 1, b_alloc, nc.vector)
                    + emit_v_units(0, 2, b_alloc, nc.vector)
                    + [lambda: emit_q_unit(0, 1, b_alloc, nc.vector)]
                    + emit_v_units(0, 3, b_alloc, nc.vector)
                    + [lambda: emit_q_unit(0, 2, b_alloc, nc.vector),
                       lambda: emit_q_unit(0, 3, b_alloc, nc.vector)]))
                load_x2(6)
                load_x2(7)
                load_x1(4)
                load_x1(5)
                emit_c_qb(0, 1, b1_kv(0) + b1_kv(1))
                emit_coll(0)
                load_x1(6)
                load_x1(7)
                emit_c_qb(0, 2, b1_kv(2) + b1_kv(3))
                emit_c_qb(0, 3, b1_q(0) + b1_q(1))
                emit_coll(1)
                emit_c_qb(1, 0, b1_q(2))
                emit_c_qb(1, 1, b1_q(3))
                emit_coll(2)
                # of loads sit on the sync queue and WAIT for their
                # collective — emit each right before its consumer so it
                # never blocks queued-behind x loads / later of loads.
                # Each D group lands >=2 qbs (~75us) after its coll fires
                # so run-to-run launch skew (10-70us observed) never
                # stalls the PE FIFO mid-stream; only coll3 stays on the
                # critical path, its wait window filled by D2 (which also
                # re-ramps the PE pstate for D3).
                emit_of_load(0)
                emit_c_qb(1, 2, emit_d_units(0))
                emit_of_load(1)
                emit_c_qb(1, 3, emit_d_units(1))
                emit_coll(3)
                _pin[0] = _pin[1]     # tail Ds go after (1,3)'s last AV
                emit_of_load(2)
                for u in emit_d_units(2):
                    u()
                emit_of_load(3)
                # strict order D3 after D2: both share the (1,3) anchor,
                # and without this edge the scheduler can interleave D3's
                # coll3-waiting matmuls among D2's, stalling the PE FIFO
                # for the whole coll3 window
                _pin[0] = _last_d[0]
                for u in emit_d_units(3):
                    u()

    nc.compile()
    return nc


def _tile_xt(x):
    """[B,N,DIM] f32 -> [TOK//NB, 128, KT, NB] bf16 block-contiguous x^T."""
    bf = ml_dtypes.bfloat16
    xt = x.reshape(TOK, DIM).T
    return np.ascontiguousarray(
        xt.reshape(KT, 128, TOK // NB, NB).transpose(2, 1, 0, 3)).astype(bf)


def make_in_maps(x1, x2, Wq, Wkv, Wproj, bproj):
    bf = ml_dtypes.bfloat16
    x1t = _tile_xt(x1)
    x2t = _tile_xt(x2)
    wq_full = Wq * (SCALE * K_SCH)   # scores arrive pre-scaled for exp
    wk_full = Wkv[:, :DIM]
    wv_full = Wkv[:, DIM:]
    wp = np.ascontiguousarray(
        Wproj.reshape(KT, 128, DIM).transpose(1, 0, 2)).astype(bf)
    bp = bproj.reshape(1, DIM).astype(bf)
    in_maps = []
    for c in range(NCORES):
        sl = slice(DLOC * c, DLOC * (c + 1))
        in_maps.append({
            "x1t": x1t, "x2t": x2t,
            "wq": np.ascontiguousarray(
                wq_full[:, sl].reshape(KT, 128, DLOC).transpose(1, 0, 2)).astype(bf),
            "wk": np.ascontiguousarray(
                wk_full[:, sl].reshape(KT, 128, DLOC).transpose(1, 0, 2)).astype(bf),
            "wv": np.ascontiguousarray(
                wv_full[:, sl].reshape(KT, 128, DLOC).transpose(1, 0, 2)).astype(bf),
            "wp": wp, "bp": bp,
        })
    return in_maps


_nc = None


def run(inputs, trace=False):
    """Returns (full_output [B,N,DIM] f32, BassKernelResults)."""
    global _nc
    from concourse.bass_utils import run_bass_kernel_spmd
    if _nc is None:
        _nc = build()
    in_maps = make_in_maps(**inputs)
    res = run_bass_kernel_spmd(_nc, in_maps, core_ids=list(range(NCORES)),
                               trace=trace)
    # unshard: core c's out rows [128j : 128j+128] are global tokens
    # 512*(2j + c//4) + 128*(c%4) .. +128
    y = np.empty((TOK, DIM), dtype=np.float32)
    for c in range(NCORES):
        oc = res.results[c]["out"]
        for j in range(NCOLL):
            g = 512 * (2 * j + c // 4) + 128 * (c % 4)
            y[g:g + 128] = oc[128 * j:128 * (j + 1)].astype(np.float32)
    return y.reshape(B, N, DIM), res


def kernel(x1, x2, Wq, Wkv, Wproj, bproj):
    y, _ = run(dict(x1=x1, x2=x2, Wq=Wq, Wkv=Wkv, Wproj=Wproj, bproj=bproj))
    return y
